# revision 16
# baseline (speedup 1.0000x reference)
"""Weighted Chamfer-MSE kernel for Trainium2 (8 NeuronCores, Bass/Tile).

Reference computes, per batch element:
    D[p, q]  = sum_c w[c]^2 * (t[p, c] - y[c, q])^2        (p=2048, q=4096)
    out      = mean_{b,p} min_q D + mean_{b,q} min_p D

Strategy (data-parallel over batch, 1 element per core):
  * Host packs the distance computation into ONE matmul with augmented
    contraction dim K=5:
        D_neg[p, q] = sum_k a[k, p] * b[k, q]
        a = [2*w2_c*t_pc (c=0..2), -wt2_p, -1],  b = [y_cq (c=0..2), 1, wy2_q]
    so D_neg = -D, and both min-reductions become max-reductions.
  * PE float32r matmuls (1 cyc/row, ~212 ns per 512-row matmul ramped)
    write D_neg fp32 into [128, 2048] PSUM tiles (TRN2 PE can only
    drain fp32).
  * The fused custom DVE op per tile maintains
       runq   = elementwise max over p-tiles   (-> min over p, per q)
       rowmax = per-row max of in0 only        (-> min over q, per p)
    At 1x (fp32, PSUM src) it costs (120+2048) cyc = 2258 ns/tile ->
    72 us/body on the DVE alone: the single-engine bottleneck.
  * Engine pipeline (all-cv, _DV_EVERY > _NPT): the otherwise-idle ACT
    engine copies EVERY PSUM tile to SBUF as **bf16** (~2130 ns/tile in
    context; rounding is 0.4% of each value's own magnitude -- measured
    ~1e-4 end-to-end rel err, since min-selection is scale-invariant to
    per-value relative noise).  The custom op carries a hand-authored
    2X_1PORT uop program (all-bf16 packed operands -> 2 elems/cycle,
    ~1127+300 ns/tile), so DVE (~46 us) and PE (27-55 us p-state
    dependent) both hide under the ACT stream (~68 us/body = ScalarE's
    54.6 us silicon streaming floor + per-op overheads; measured equal
    to an ACT-only ablation).  The engine falls back to the regular 1x
    program (still correct) whenever an operand pattern doesn't qualify
    for 2x.  Splitting ANY tile's PSUM reads onto the DVE instead was
    measured slower in every topology (ACT/DVE PSUM read contention).
  * Tail (once, not per body): the bf16 running max is transposed
    directly on the PE (bf16 identity, bf16 PSUM out) for the two
    q-chunks' 128-partition reductions, one DVE reduce each; the
    rowmax columns reduce to one value per p-tile.  With _DV_EVERY <=
    _NPT a direct-fp32 chain also exists and is max-merged here.
    Final sums happen on the host from the small DMA'd staging tile.
"""

import os
import numpy as np
from contextlib import ExitStack

from concourse import bacc, bass, tile, mybir
from concourse.bass_utils import run_bass_kernel_spmd
from concourse.masks import make_identity
from concourse.dve_spec import Spec, Src0, Src1, MaxNeg, maxx, lower
from concourse.dve_uop import (
    AluInp, AluOp, DelayInp, DveOpSpec, InpSel, OutPath, OutSel, Trigger,
    UopConfig, UopDpConfig, ENABLE, DISABLE,
)
from concourse.dve_ops import (
    DveOp, OPS, has_src1, CUSTOM_DVE_SPECS,
    _SUB_OPCODE_FOR_NAME, _CUSTOM_DVE_ROW_BASE, _COMPILE_CACHE,
    get_dve_sub_opcode,
)

_B, _C, _H, _W = 8, 3, 64, 64
_P = 2048
_Q = _H * _W  # 4096
_K = 5
_NCORES = 8
_F32 = mybir.dt.float32
_BF16 = mybir.dt.bfloat16
_NEG = -3.0e38

_CHUNK = 2048            # q-chunk of the runq buffers
_NCHUNK = _Q // _CHUNK   # 2
_FD = 2048               # PSUM tile free dim: [128, 2048] fp32 = 4 banks
_NH = _CHUNK // _FD      # 2 sub-tiles (halves) per (chunk, ptile)
_NPT = _P // 128         # 16 stationary tiles
_NMM = _FD // 512        # 2 matmuls per sub-tile: 512 fp32 = 1 bank
_NBLK = _CHUNK // 128    # 16 transpose blocks per chunk in the tail
_DV_EVERY = 10 ** 6      # direct-fp32-path period; >16: every tile goes ACT->bf16->2x DVE
_NDUM = 0                # dummy matmuls per sub-tile (0 = disabled; >0 tripped NRT_EXEC_UNIT_UNRECOVERABLE)

# Stashed BassKernelResults from the most recent kernel() call (for test.py).
LAST_RESULTS = None


def _chamfer_uops_2x(uops_1x):
    """Hand-authored 2X_1PORT program (2 uops, mirroring the regular FSM).

    Input mux lanes -> delay chains at block 0:
      chain0=SRC_0  chain1=SRC_1  chain2=MAX_NEG  chain3=SRC_0_HI
      chain4=SRC_1_HI
    Blocks:
      dp0: lo  = MAX(SRC_0, SRC_1)
      dp1: hi  = MAX(SRC_0_HI, SRC_1_HI); chain1 <- lo (SRC_1 dead here)
      dp2: t   = MAX(SRC_0, SRC_0_HI);    chain4 <- hi (SRC_1_HI dead here)
      dp3: acc = MAX(acc, t)  (uop0: BYPASS MAX_NEG seed, like the 1x init)
      dp4-7: pass the accum value along the ALU chain (A-path), carry
             chains 1 (lo) and 4 (hi) to the out mux.
    Out mux (steady): WR0_LO <- DELAY_1 (lo), WR0_HI <- DELAY_4 (hi).
    uop[0] is a pure init cycle (consumes no input, writes no output,
    seeds the accum chain with MAX_NEG), exactly like the regular
    program's uop[0]."""
    inp = [InpSel.ZERO, InpSel.SRC_0, InpSel.SRC_1, InpSel.MAX_NEG,
           InpSel.SRC_0_HI, InpSel.SRC_1_HI, InpSel.ZERO, InpSel.ZERO]
    inp_enable = [0, 1, 1, 1, 1, 1, 0, 0]

    def dp_chain(keep, rewrite=None):
        d = UopDpConfig()
        for c in keep:
            d.delay[c] = DelayInp.PREV_DELAY
            d.delay_enable[c] = ENABLE
        if rewrite is not None:
            d.delay[rewrite] = DelayInp.PREV_ALU_OUT
            d.delay_enable[rewrite] = ENABLE
        return d

    def body_dps(init):
        dps = []
        d0 = dp_chain([0, 1, 2, 3, 4])
        d0.enable_alu(AluOp.MAX, AluInp.PREV_DELAY_0, AluInp.PREV_DELAY_1)
        dps.append(d0)
        d1 = dp_chain([0, 2, 3, 4], rewrite=1)
        d1.enable_alu(AluOp.MAX, AluInp.PREV_DELAY_3, AluInp.PREV_DELAY_4)
        dps.append(d1)
        d2 = dp_chain([1, 2], rewrite=4)
        d2.enable_alu(AluOp.MAX, AluInp.PREV_DELAY_0, AluInp.PREV_DELAY_3)
        dps.append(d2)
        d3 = dp_chain([1, 4] + ([2] if init else []))
        if init:
            # seed the accum chain with MAX_NEG (chain 2), as the 1x
            # program's init uop does
            d3.enable_alu(AluOp.BYPASS, AluInp.PREV_DELAY_2)
        else:
            d3.enable_alu(AluOp.MAX, AluInp.CURR_ALU_OUT, AluInp.PREV_ALU_OUT)
        d3.alu_out_a_enable = ENABLE
        dps.append(d3)
        for _ in range(4):
            dk = dp_chain([1, 4])
            dk.pass_through_alu()
            dk.alu_out_a_enable = ENABLE
            dps.append(dk)
        return dps

    u0 = UopConfig(
        inp=list(inp), inp_enable=list(inp_enable),
        trigger=(Trigger.COUNT, Trigger.NONE, Trigger.NONE),
        repeat_count=1, next_uop=(1, 0, 0), accum_enabled=ENABLE,
        datapath_config=body_dps(init=True),
    )
    u1 = UopConfig(
        inp=list(inp), inp_enable=list(inp_enable),
        out={OutPath.WR0_LO: OutSel.DELAY_1, OutPath.WR0_HI: OutSel.DELAY_4,
             OutPath.WR1_LO: OutSel.ALU_OUT, OutPath.WR1_HI: OutSel.ALU_OUT},
        out_enable={OutPath.WR0_LO: ENABLE, OutPath.WR0_HI: ENABLE,
                    OutPath.WR1_LO: DISABLE, OutPath.WR1_HI: DISABLE},
        require_inp0=ENABLE, require_inp1=ENABLE,
        trigger=(Trigger.SRC_TENSOR_DONE, Trigger.NONE, Trigger.NONE),
        next_uop=(0, 0, 0), accum_enabled=ENABLE,
        datapath_config=body_dps(init=False),
    )
    return [u0, u1]


class _HandEditedMaxMaxOp(DveOp):
    """Custom fused DVE op (uops ship inside the NEFF, no firmware dep):
        out[p,k]     = max(in0[p,k], in1[p,k])   -- elementwise accumulate
        accum_out[p] = max_k in0[p,k]            -- row reduce of in0 ONLY
    The native TENSOR_TENSOR_REDUCE opcode reduces the body output (which
    would contaminate the row max with in1's history), so the steady-state
    1x uop is hand-edited: the accum ALU's operand B is repointed from the
    body output to the raw Src0 delay lane.  A hand-authored 2X_1PORT
    variant (see _chamfer_uops_2x) gives 2 elems/cycle when all operands
    are packed bf16; the engine falls back to the 1x program otherwise."""

    def compile(self, ver):
        key = (self.name, ver)
        if (r := _COMPILE_CACHE.get(key)) is not None:
            return r
        uops = lower(self.spec, ver=ver)
        assert len(uops) == 2
        uops[1].datapath_config[1].alu_src1 = AluInp.PREV_DELAY_0
        r = DveOpSpec(
            name=self.name,
            opcode=get_dve_sub_opcode(self.name),
            uops=uops,
            uops_2x=_chamfer_uops_2x(uops),
            perf_max=1,
            rd1_en=has_src1(self.spec),
        )
        _COMPILE_CACHE[key] = r
        return r


def _register_chamfer_op():
    name = "CHAMFER_MAX_SRC0MAX"
    if name in _SUB_OPCODE_FOR_NAME:
        return next(op for op in OPS if op.name == name)
    spec = Spec(
        body=maxx(Src0, Src1),
        accum=maxx,
        accum_init=MaxNeg,
        reference=lambda in0, in1, c0, c1, c2: (
            np.maximum(in0, in1),
            in0.max(axis=-1, keepdims=True),
        ),
    )
    _SUB_OPCODE_FOR_NAME[name] = _CUSTOM_DVE_ROW_BASE + len(OPS)
    op = _HandEditedMaxMaxOp(name, spec, subdim=False, uops_sha={})
    OPS.append(op)
    CUSTOM_DVE_SPECS[name] = spec
    return op


_CHAMFER_OP = _register_chamfer_op()


def _emit_chamfer(nc, out, in0, in1, accum_out):
    inst = nc.vector._custom_dve(
        _CHAMFER_OP, out=out, in0=in0, in1=in1, accum_out=accum_out,
    )
    # byte-36 bits 7:6: highest engine-reachable perf slot (1 = 2X_1PORT).
    # The engine checks operand patterns at runtime and silently falls
    # back to the 1x program when they don't qualify.
    try:
        inst.ins.perf_max = 1
    except AttributeError:
        pass  # older InstCustomDveAnt: runs the (correct) 1x program
    return inst


def _build_nc(repeat=1, nloop=1):
    """repeat: python-unrolled bodies (bodies are self-initializing).
    nloop: hardware For_i trip count around them -- total bodies
    executed = repeat * nloop."""
    nc = bacc.Bacc("TRN2", target_bir_lowering=False, debug=False)
    ab_dram = nc.dram_tensor(
        "ab", [_K, _P + _Q], mybir.dt.float32r, kind="ExternalInput"
    ).ap()
    # out columns: 0:16 rowmax-combined (p-side, [part, ptile]); 16:32
    # chunk-0 blockmax; 32:48 chunk-1 blockmax (q-side colmax).  Host sums.
    out_dram = nc.dram_tensor("out", [128, 48], _F32, kind="ExternalOutput").ap()

    with ExitStack() as ctx:
        tc = ctx.enter_context(tile.TileContext(nc))
        sbuf = ctx.enter_context(tc.tile_pool(name="sbuf", bufs=1))
        cvp = ctx.enter_context(tc.tile_pool(name="cv", bufs=6))
        _tile_banks = _FD // 512
        _psum_bufs = (8 - (1 if _NDUM else 0)) // _tile_banks
        psum = ctx.enter_context(
            tc.tile_pool(name="psum", bufs=_psum_bufs, space="PSUM"))
        dpsum = ctx.enter_context(tc.tile_pool(name="dpsum", bufs=1, space="PSUM"))

        ab_sb = sbuf.tile([_K, _P + _Q], mybir.dt.float32r)
        # Chunked input DMA: the first transfer carries a plus the first
        # q-chunk of b so chunk-0 matmuls start before the full load lands.
        nc.sync.dma_start(ab_sb[:, 0:_P + _CHUNK], ab_dram[:, 0:_P + _CHUNK])
        for c in range(1, _NCHUNK):
            q0 = _P + c * _CHUNK
            nc.sync.dma_start(ab_sb[:, q0:q0 + _CHUNK], ab_dram[:, q0:q0 + _CHUNK])

        # With _DV_EVERY > _NPT no tile takes the direct fp32 path, so the
        # fp32 chain (negf/runf/rmf + their init memsets and tail merges)
        # is dead weight -- skip it entirely to trim single-shot latency.
        all_cv = _DV_EVERY > _NPT

        # Constant -inf tiles: in1 of each chain's first DVE op, so the
        # running buffers need no per-body memset.
        negb = sbuf.tile([128, _CHUNK], _BF16)
        nc.gpsimd.memset(negb[:], _NEG)
        ident = sbuf.tile([128, 128], _BF16 if all_cv else _F32)
        make_identity(nc, ident[:])

        runb = sbuf.tile([128, _Q], _BF16)   # bf16 chain (ACT-converted tiles)
        rmb = sbuf.tile([128, _NPT * _NCHUNK * _NH], _BF16)
        nc.gpsimd.memset(rmb[:], _NEG)
        negf = runf = rmf = None
        if not all_cv:
            negf = sbuf.tile([128, _CHUNK], _F32)
            nc.gpsimd.memset(negf[:], _NEG)
            # the tail max-merges runb into runf; columns no direct op
            # writes must start at -inf, not garbage
            runf = sbuf.tile([128, _Q], _F32)
            nc.gpsimd.memset(runf[:], _NEG)
            rmf = sbuf.tile([128, _NPT * _NCHUNK * _NH], _F32)
            nc.gpsimd.memset(rmf[:], _NEG)

        dum = dpsum.tile([128, 512], _F32, tag="dum") if _NDUM else None

        def body():
            first_f = [[True] * _NH for _ in range(_NCHUNK)]
            first_b = [[True] * _NH for _ in range(_NCHUNK)]
            for c in range(_NCHUNK):
                for pi in range(_NPT):
                    lhsT = ab_sb[:, pi * 128:(pi + 1) * 128]
                    for h in range(_NH):
                        qof = c * _CHUNK + h * _FD
                        qf = runf[:, qof:qof + _FD] if not all_cv else None
                        qb = runb[:, qof:qof + _FD]
                        pt = psum.tile([128, _FD], _F32, tag="pt")
                        for qk in range(_NMM):
                            q0 = _P + qof + qk * 512
                            # float32r: full-rate fp32 (plain fp32 = 4 cyc/row)
                            nc.tensor.matmul(
                                pt[:, qk * 512:(qk + 1) * 512],
                                lhsT,
                                ab_sb[:, q0:q0 + 512],
                                start=True,
                                stop=True,
                            )
                        for _ in range(_NDUM):
                            # filler matmul into the sacrificial bank: keeps
                            # the PE executing through consumer stalls so its
                            # p-state clock stays ramped (idle resets it)
                            nc.tensor.matmul(
                                dum[:], lhsT, ab_sb[:, _P:_P + 512],
                                start=True, stop=True,
                            )
                        col = (pi * _NCHUNK + c) * _NH + h
                        if pi % _DV_EVERY == _DV_EVERY - 1:
                            # direct fp32 path, 1x
                            _emit_chamfer(
                                nc, out=qf, in0=pt[:],
                                in1=negf[:] if first_f[c][h] else qf,
                                accum_out=rmf[:, col:col + 1],
                            )
                            first_f[c][h] = False
                        else:
                            # ACT converts to packed bf16 in SBUF; fused op
                            # runs its 2X_1PORT program
                            cv = cvp.tile([128, _FD], _BF16, tag="cv")
                            nc.scalar.activation(
                                cv[:], pt[:], mybir.ActivationFunctionType.Copy,
                            )
                            _emit_chamfer(
                                nc, out=qb, in0=cv[:],
                                in1=negb[:] if first_b[c][h] else qb,
                                accum_out=rmb[:, col:col + 1],
                            )
                            first_b[c][h] = False

        if nloop > 1:
            with tc.For_i(0, nloop, 1):
                for _ in range(repeat):
                    body()
        else:
            for _ in range(repeat):
                body()

        # Tail (once): per-chunk partition-max of the running max via PE
        # transposes + one DVE reduce each; rowmax combined across dtype
        # chains and chunk columns.  Under all_cv the transposes read the
        # bf16 chain directly (bf16 transpose = 1 cyc/row, bf16 PSUM out),
        # skipping the [128,4096] fp32 staging copy.
        if all_cv:
            tsrc = runb
        else:
            scr = sbuf.tile([128, _Q], _F32)
            nc.vector.tensor_copy(scr[:], runb[:])
            nc.vector.tensor_tensor(scr[:], scr[:], runf[:], mybir.AluOpType.max)
            tsrc = scr

        stage = sbuf.tile([128, 48], _F32)
        rmscr = sbuf.tile([128, _NPT * _NCHUNK * _NH], _F32)
        nc.vector.tensor_copy(rmscr[:], rmb[:])
        if not all_cv:
            nc.vector.tensor_tensor(rmscr[:], rmscr[:], rmf[:], mybir.AluOpType.max)
        nc.vector.tensor_reduce(
            stage[:, 0:_NPT],
            rmscr[:].rearrange("p (a b) -> p a b", b=_NCHUNK * _NH),
            axis=mybir.AxisListType.X,
            op=mybir.AluOpType.max,
        )
        if _NDUM:
            # consume the sacrificial bank so its matmuls aren't dead code
            nc.vector.tensor_reduce(
                stage[:, 16:17], dum[:].rearrange("p (a b) -> p a b", b=512),
                axis=mybir.AxisListType.X, op=mybir.AluOpType.max,
            )
        for c in range(_NCHUNK):
            for h in range(_NH):
                qs = tsrc[:, c * _CHUNK + h * _FD:c * _CHUNK + (h + 1) * _FD]
                tp = psum.tile([128, _FD], _BF16 if all_cv else _F32, tag="pt")
                nb = _FD // 128
                for j in range(nb):
                    nc.tensor.transpose(
                        tp[:, j * 128:(j + 1) * 128],
                        qs[:, j * 128:(j + 1) * 128],
                        ident[:],
                    )
                col0 = 16 + (c * _NH + h) * nb
                nc.vector.tensor_reduce(
                    stage[:, col0:col0 + nb],
                    tp[:].rearrange("p (j b) -> p j b", b=128),
                    axis=mybir.AxisListType.X,
                    op=mybir.AluOpType.max,
                )
        nc.sync.dma_start(out_dram[:], stage[:])
    nc.compile()
    return nc


def _pack_inputs(y, t, weights):
    """Build per-core augmented factor matrices (numpy, O((p+q)*c) per core)."""
    w2 = (weights * weights).astype(np.float32)
    in_maps = []
    for i in range(_NCORES):
        yq = y[i].reshape(_C, _Q)
        ti = t[i]
        a = np.empty((_K, _P), np.float32)
        a[0:_C] = (2.0 * w2)[:, None] * ti.T
        a[_C] = -(w2[None, :] * ti * ti).sum(axis=1)
        a[_C + 1] = -1.0
        ab = np.empty((_K, _P + _Q), np.float32)
        ab[:, :_P] = a
        ab[0:_C, _P:] = yq
        ab[_C, _P:] = 1.0
        ab[_C + 1, _P:] = (w2[:, None] * yq * yq).sum(axis=0)
        in_maps.append({"ab": ab})
    return in_maps


def _combine(results):
    """Host-side reduction of the per-core output tensors to the scalar."""
    bp_neg = 0.0
    bq_neg = 0.0
    for r in results:
        o = np.asarray(r["out"], dtype=np.float64)
        bp_neg += o[:, 0:_NPT].sum()
        bq_neg += o[:, 16:16 + _NCHUNK * _NBLK].sum()
    return -(bp_neg / (_B * _P) + bq_neg / (_B * _Q))


def kernel(y, t, weights):
    global LAST_RESULTS
    y = np.asarray(y, dtype=np.float32)
    t = np.asarray(t, dtype=np.float32)
    weights = np.asarray(weights, dtype=np.float32)
    assert y.shape == (_B, _C, _H, _W) and t.shape == (_B, _P, _C)

    in_maps = _pack_inputs(y, t, weights)
    nc = _build_nc()
    trace = bool(os.environ.get("BASS_CHAMFER_TRACE"))
    res = run_bass_kernel_spmd(
        nc, in_maps, core_ids=list(range(_NCORES)), trace=trace
    )
    LAST_RESULTS = res
    return np.float32(_combine(res.results))
